# revision 12
# baseline (speedup 1.0000x reference)
"""Deformable attention module on Trainium2 (Bass/Tile), 8-core data-parallel.

v2 strategy (per core = one batch):
  1. Phase A (8 chunks of 16 y-rows): load fm [C=256, HW] f32 -> SBUF;
     PE-transpose 128x128 blocks, cast bf16; write TWO y-banded copies of
     the transposed map to DRAM (A: bands (2b,2b+1), B: (2b+1,2b+2)) so a
     2x2 bilinear patch is ONE contiguous 2KB gather element.  Fused into
     the same loop: q_featT = bilinear sample at the fixed ref grid,
     computed as PE selection-matmuls (fmT_chunk^T @ SXw) accumulating the
     two y-rows of each ref row in PSUM -> no separate q-feat gather, no
     DRAM round trip, available the moment the last chunk is transposed.
  2. Offset MLP batched over all 8 query blocks (PE matmuls, manual
     layernorm, composed tanh-gelu) + queries -> overlaps phase A's banded
     write drain.
  3. Coordinates -> robust floor -> clip -> bilinear weights + banded patch
     indices (int16); identity-slice PE matmuls wrap indices into the
     dma_gather layout.
  4. Per query-block g: one 2-half dma_gather of 1024 2KB patches (GPSIMD
     runs only descriptor generation, so gathers pipeline across blocks).
     Per point: DVE builds the bilinear diagonal, PE diag-transpose-combine
     -> sT [C, 128] bf16, K/V matmuls -> interleaved K/V planes.  Scores:
     stacked q*k multiply + two-stage segmented reduce.  Softmax, attn*V on
     DVE + bf16 tree-sum -> out.
"""

import sys

for _p in ("/opt/trn_rl_repo", "/root/.axon_site/_ro/trn_rl_repo"):
    if _p not in sys.path:
        sys.path.append(_p)

import numpy as np
import ml_dtypes

import concourse.bass as bass
import concourse.bacc as bacc
import concourse.tile as tile
from concourse import mybir

F32 = mybir.dt.float32
BF16 = mybir.dt.bfloat16
I16 = mybir.dt.int16
I32 = mybir.dt.int32

C = 256
H = W = 128
HW = H * W
NQ = 1024          # (H//4) * (W//4)
NHEAD = 8
DH = 32
NG = 8             # query blocks of 128
NP = 8             # sampling points per query (= NHEAD)

# banded gather source: "unit" = 512 bf16 = 2 image rows; element = 2 units
UNIT = 512                 # bf16 elems per step unit
B_BASE = 8192              # B copy starts at unit 8192
G_UNITS = 16448            # 16384 + pad

_BF = ml_dtypes.bfloat16


def _ref_grids():
    """Per-ref-point pixel coords / floor / weights, matching reference.py fp32 math."""
    c = np.linspace(-1.0, 1.0, 32).astype(np.float32)
    pix = ((c + 1.0) * 0.5 * (W - 1)).astype(np.float32)   # [32]
    p0 = np.clip(np.floor(pix), 0.0, W - 2).astype(np.float32)
    wf = np.clip(pix - p0, 0.0, 1.0).astype(np.float32)
    return pix, p0, wf


def _host_constants():
    pix, p0, wf = _ref_grids()

    # selection matmul rhs: SXw[x, (gy_local, row), gx] built per 16-row
    # chunk; all chunks share the same (x0, wx) column structure and the
    # same (gy_local -> wy) pattern EXCEPT wy varies per gy, so store the
    # full [128, 32*2, 32] table indexed by (gy, row).
    x0 = p0.astype(np.int32)
    wx = wf
    y0 = p0.astype(np.int32)          # same grid in y
    wy = wf
    SXw = np.zeros((128, 64, 32), np.float32)
    for gy in range(32):
        for r in range(2):
            wyfac = (1.0 - wy[gy]) if r == 0 else wy[gy]
            for gx in range(32):
                SXw[x0[gx], gy * 2 + r, gx] += wyfac * (1.0 - wx[gx])
                SXw[x0[gx] + 1, gy * 2 + r, gx] += wyfac * wx[gx]
    SXw = SXw.astype(_BF)

    ident = np.eye(128, dtype=np.float32)
    # interleaved identity: ident4[r, m*4+wi] = (r == m), so the diag build's
    # weight operand has a packed (stride-1) last dim -> DVE 2x mode
    ident4 = np.repeat(np.eye(128, dtype=np.float32), 4, axis=1).astype(_BF)  # [128,512]
    # fold8[r, t, m] = 1 iff r == t*16 + (m % 16): the identity-slice matmul
    # then emits idx row block t into all 8 replicated 16-partition groups.
    fold8 = np.zeros((128, 8, 128), np.float32)
    for t in range(8):
        for m in range(128):
            fold8[t * 16 + (m % 16), t, m] = 1.0
    fold8 = fold8.reshape(128, 1024)

    # ref pixel coords in the [128 r, 8 g] layout (n = g*128 + r)
    refx = pix[np.arange(128) % 32].astype(np.float32)[:, None]          # [128,1]
    g_idx, r_idx = np.meshgrid(np.arange(NG), np.arange(128), indexing="xy")
    refy = pix[(g_idx * 4 + r_idx // 32)].astype(np.float32)             # [128,8]
    return dict(
        ident=ident, ident4=ident4, fold8=fold8, SXw=SXw,
        refx=refx, refy=refy,
    )


def build_nc(debug: bool = False):
    nc = bacc.Bacc()

    fm = nc.declare_dram_parameter("fm", [C, HW], F32, isOutput=False)
    Wqs = nc.declare_dram_parameter("Wqs", [C, C], F32, isOutput=False)
    WkvT = nc.declare_dram_parameter("WkvT", [C, 2 * C], BF16, isOutput=False)
    Wo1 = nc.declare_dram_parameter("Wo1", [C, 64], F32, isOutput=False)
    Wo2s = nc.declare_dram_parameter("Wo2s", [64, 16], F32, isOutput=False)
    bo1b = nc.declare_dram_parameter("bo1b", [128, 64], F32, isOutput=False)
    bo2b = nc.declare_dram_parameter("bo2b", [128, 16], F32, isOutput=False)
    lngb = nc.declare_dram_parameter("lngb", [128, 64], F32, isOutput=False)
    lnbb = nc.declare_dram_parameter("lnbb", [128, 64], F32, isOutput=False)
    identP = nc.declare_dram_parameter("ident", [128, 128], F32, isOutput=False)
    ident4P = nc.declare_dram_parameter("ident4", [128, 512], BF16, isOutput=False)
    SXwP = nc.declare_dram_parameter("SXw", [128, 64, 32], BF16, isOutput=False)
    refxP = nc.declare_dram_parameter("refx", [128, 1], F32, isOutput=False)
    refyP = nc.declare_dram_parameter("refy", [128, NG], F32, isOutput=False)
    fold8P = nc.declare_dram_parameter("fold8", [128, 1024], F32, isOutput=False)

    out = nc.declare_dram_parameter("out", [NQ, C], F32, isOutput=True)
    dbg = {}
    if debug:
        dbg["qfT"] = nc.declare_dram_parameter("d_qfT", [2, 128, NQ], F32, isOutput=True)
        dbg["off"] = nc.declare_dram_parameter("d_off", [128, NG, 16], F32, isOutput=True)
        dbg["xy"] = nc.declare_dram_parameter("d_xy", [128, 2, 64], F32, isOutput=True)
        dbg["w4"] = nc.declare_dram_parameter("d_w4", [128, 256], F32, isOutput=True)
        dbg["idxf"] = nc.declare_dram_parameter("d_idxf", [128, 64], F32, isOutput=True)
        dbg["q"] = nc.declare_dram_parameter("d_q", [128, NG, C], F32, isOutput=True)

    with tile.TileContext(nc) as tc, tc.tile_pool(name="main", bufs=1) as main, \
         tc.tile_pool(name="consts", bufs=1) as consts, \
         tc.tile_pool(name="dram", bufs=1, space="DRAM") as dram:

        # ---- constants to SBUF (scalar queue; sync reserved for fm loads) ----
        ident_sb = consts.tile([128, 128], F32)
        nc.scalar.dma_start(out=ident_sb[:], in_=identP[:])
        SXw_sb = consts.tile([128, 64, 32], BF16)
        nc.scalar.dma_start(out=SXw_sb[:], in_=SXwP[:])
        Wo1_sb = consts.tile([128, 2, 64], F32)
        nc.scalar.dma_start(out=Wo1_sb[:], in_=Wo1.rearrange("(ch k) d -> k ch d", ch=2))
        Wqs_sb = consts.tile([128, 2, C], F32)
        nc.scalar.dma_start(out=Wqs_sb[:], in_=Wqs.rearrange("(ch k) d -> k ch d", ch=2))
        bo1_sb = consts.tile([128, 64], F32)
        nc.scalar.dma_start(out=bo1_sb[:], in_=bo1b[:])
        bo2_sb = consts.tile([128, 16], F32)
        nc.scalar.dma_start(out=bo2_sb[:], in_=bo2b[:])
        lng_sb = consts.tile([128, 64], F32)
        nc.scalar.dma_start(out=lng_sb[:], in_=lngb[:])
        lnb_sb = consts.tile([128, 64], F32)
        nc.scalar.dma_start(out=lnb_sb[:], in_=lnbb[:])
        Wo2_sb = consts.tile([64, 16], F32)
        nc.scalar.dma_start(out=Wo2_sb[:], in_=Wo2s[:])
        ident4_sb = consts.tile([128, 512], BF16)
        nc.scalar.dma_start(out=ident4_sb[:], in_=ident4P[:])
        fold8_sb = consts.tile([128, 1024], F32)
        nc.scalar.dma_start(out=fold8_sb[:], in_=fold8P[:])
        refx_sb = consts.tile([128, 1], F32)
        nc.scalar.dma_start(out=refx_sb[:], in_=refxP[:])
        refy_sb = consts.tile([128, NG], F32)
        nc.scalar.dma_start(out=refy_sb[:], in_=refyP[:])
        WkvT_sb = consts.tile([128, 2, 2 * C], BF16)
        nc.scalar.dma_start(out=WkvT_sb[:], in_=WkvT.rearrange("(ch k) d -> k ch d", ch=2))
        eps_sb = consts.tile([128, 1], F32)
        nc.vector.memset(eps_sb[:], 1e-5)
        # preload activation tables (Sqrt/Tanh/Exp) off the critical path
        tdummy = consts.tile([128, 1], F32)
        nc.vector.memset(tdummy[:], 1.0)
        for fn in (mybir.ActivationFunctionType.Sqrt,
                   mybir.ActivationFunctionType.Tanh,
                   mybir.ActivationFunctionType.Exp):
            nc.scalar.activation(out=tdummy[:], in_=tdummy[:], func=fn)

        # ---- phase A: fm -> banded transposed copies A/B in DRAM,
        #      fused q_featT selection matmuls ----
        G = dram.tile([G_UNITS, UNIT], BF16)
        Gt = G[:].tensor

        def g_ap(unit_off, elem_off, dims):
            return bass.AP(tensor=Gt, offset=unit_off * UNIT + elem_off, ap=list(dims))

        qfT_sb = main.tile([128, 2, NQ], F32)      # [c_lo/c_hi part, ch, n]
        q_sb = main.tile([128, NG, C], BF16)
        off_sb = main.tile([128, NG, 16], F32)
        _, p0_grid, _ = _ref_grids()
        with tc.tile_pool(name="pA_fm", bufs=2) as pA_fm, \
             tc.tile_pool(name="pA_ps", bufs=3, space="PSUM") as pA_ps, \
             tc.tile_pool(name="pA_psq", bufs=2, space="PSUM") as pA_psq, \
             tc.tile_pool(name="pC_ps", bufs=2, space="PSUM") as pC_ps, \
             tc.tile_pool(name="pC", bufs=2) as pC, \
             tc.tile_pool(name="pA_out", bufs=3) as pA_out:
            # zero-fill regions the banded writes never touch (B band 63
            # upper halves + tail pad) so the gather source is fully defined
            zpad = pA_out.tile([128, UNIT], BF16, tag="zpad")
            nc.vector.memset(zpad[:], 0.0)
            nc.gpsimd.dma_start(out=G[:][B_BASE + 8064:B_BASE + 8192, C:2 * C],
                                in_=zpad[:, 0:C])
            nc.gpsimd.dma_start(out=G[:][2 * B_BASE:2 * B_BASE + 64, :],
                                in_=zpad[0:64, :])
            for k in range(8):             # 16-row chunks
                fm_sb = pA_fm.tile([128, 2, HW // 8], F32, tag="fm_sb")
                hw0 = k * (HW // 8)
                nc.sync.dma_start(out=fm_sb[:, 0, :], in_=fm[0:128, hw0:hw0 + HW // 8])
                nc.sync.dma_start(out=fm_sb[:, 1, :], in_=fm[128:256, hw0:hw0 + HW // 8])
                # cast f32 -> bf16 (split across DVE / ACT), then X-bar
                # DMA-transpose straight into the fmT layout: in [c, 16y*128x]
                # col-tile t = y, so out[x, y, c] per ch half.
                fmb_sb = pA_fm.tile([128, 2, HW // 8], BF16, tag="fmb_sb")
                nc.vector.tensor_copy(out=fmb_sb[:, 0, :], in_=fm_sb[:, 0, :])
                nc.scalar.copy(out=fmb_sb[:, 1, :], in_=fm_sb[:, 1, :])
                fmT_sb = pA_out.tile([128, 16, C], BF16, tag="fmT_sb")
                nc.sync.dma_start_transpose(out=fmT_sb[:, :, 0:128], in_=fmb_sb[:, 0, :])
                nc.scalar.dma_start_transpose(out=fmT_sb[:, :, 128:256], in_=fmb_sb[:, 1, :])
                # banded writes for this chunk (scalar + gpsimd queues; sync
                # stays pure-load so chunk k+1 loads are never stuck behind
                # a write waiting on chunk k's transposes)
                band0 = 8 * k                  # first A band of this chunk
                dstA = g_ap(band0 * 128, 0,
                            [[UNIT, 128], [128 * UNIT, 8], [1, 2 * C]])
                srcA = bass.AP(tensor=fmT_sb[:].tensor,
                               offset=fmT_sb[:].offset,
                               ap=[fmT_sb[:].ap[0], [2 * C, 8], [1, 2 * C]])
                nc.scalar.dma_start(out=dstA, in_=srcA)
                dstB = g_ap(B_BASE + band0 * 128, 0,
                            [[UNIT, 128], [128 * UNIT, 7], [1, 2 * C]])
                srcB = bass.AP(tensor=fmT_sb[:].tensor,
                               offset=fmT_sb[:].offset + C,
                               ap=[fmT_sb[:].ap[0], [2 * C, 7], [1, 2 * C]])
                nc.gpsimd.dma_start(out=dstB, in_=srcB)
                dstBt = g_ap(B_BASE + (band0 + 7) * 128, 0,
                             [[UNIT, 128], [1, C]])
                srcBt = bass.AP(tensor=fmT_sb[:].tensor,
                                offset=fmT_sb[:].offset + 15 * C,
                                ap=[fmT_sb[:].ap[0], [1, C]])
                nc.gpsimd.dma_start(out=dstBt, in_=srcBt)
                if band0 > 0:
                    dstBh = g_ap(B_BASE + (band0 - 1) * 128, C,
                                 [[UNIT, 128], [1, C]])
                    srcBh = bass.AP(tensor=fmT_sb[:].tensor,
                                    offset=fmT_sb[:].offset,
                                    ap=[fmT_sb[:].ap[0], [1, C]])
                    nc.gpsimd.dma_start(out=dstBh, in_=srcBh)
                # q_featT selection matmuls: chunk k holds ref rows
                # gy = 4k..4k+3 with y0(gy), y0(gy)+1 inside the chunk.
                psq = pA_psq.tile([128, 2, 128], F32, tag="psq")
                for gl in range(4):
                    gy = 4 * k + gl
                    y0 = int(p0_grid[gy])
                    j0 = y0 - 16 * k
                    for ch in range(2):
                        for r in range(2):
                            nc.tensor.matmul(
                                out=psq[:, ch, gl * 32:(gl + 1) * 32],
                                lhsT=fmT_sb[:, j0 + r, ch * 128:(ch + 1) * 128],
                                rhs=SXw_sb[:, gy * 2 + r, :],
                                start=(r == 0), stop=(r == 1),
                            )
                if k % 2 == 0:
                    nc.vector.tensor_copy(out=qfT_sb[:, :, k * 128:(k + 1) * 128],
                                          in_=psq[:])
                else:
                    nc.scalar.copy(out=qfT_sb[:, :, k * 128:(k + 1) * 128],
                                   in_=psq[:])

                # ---- offset MLP + queries for block k (fused into phase A) ----
                # one PSUM bank carved into regions: [0:64) ps_h, [64:192)
                # ps_t (64 partitions), [192:208) ps_off, [208:464) ps_q
                pCall = pC_ps.tile([128, 512], F32, tag="ps_c")
                ps_h = pCall[:, 0:64]
                ps_t = pCall[0:64, 64:192]
                ps_off = pCall[:, 192:208]
                ps_q = pCall[:, 208:464]
                for ch in range(2):
                    nc.tensor.matmul(out=ps_h,
                                     lhsT=qfT_sb[:, ch, k * 128:(k + 1) * 128],
                                     rhs=Wo1_sb[:, ch, :],
                                     start=(ch == 0), stop=(ch == 1))
                h_sb = pC.tile([128, 64], F32, tag="h_sb")
                nc.vector.tensor_add(h_sb[:], ps_h, bo1_sb[:])
                # manual layernorm over the last (64) axis
                sq = pC.tile([128, 64], F32, tag="sq")
                nc.vector.tensor_mul(sq[:], h_sb[:], h_sb[:])
                mu = pC.tile([128, 1], F32, tag="mu")
                nc.vector.tensor_reduce(out=mu[:], in_=h_sb[:],
                                        axis=mybir.AxisListType.X, op=mybir.AluOpType.add)
                ex2 = pC.tile([128, 1], F32, tag="ex2")
                nc.vector.tensor_reduce(out=ex2[:], in_=sq[:],
                                        axis=mybir.AxisListType.X, op=mybir.AluOpType.add)
                nc.vector.tensor_scalar(out=mu[:], in0=mu[:], scalar1=1.0 / 64, scalar2=None,
                                        op0=mybir.AluOpType.mult)
                nc.vector.tensor_scalar(out=ex2[:], in0=ex2[:], scalar1=1.0 / 64, scalar2=None,
                                        op0=mybir.AluOpType.mult)
                mu2 = pC.tile([128, 1], F32, tag="mu2")
                nc.vector.tensor_mul(mu2[:], mu[:], mu[:])
                var = pC.tile([128, 1], F32, tag="var")
                nc.vector.tensor_sub(var[:], ex2[:], mu2[:])
                sd = pC.tile([128, 1], F32, tag="sd")
                nc.scalar.activation(out=sd[:], in_=var[:],
                                     func=mybir.ActivationFunctionType.Sqrt,
                                     bias=eps_sb[:])
                rstd = pC.tile([128, 1], F32, tag="rstd")
                nc.vector.reciprocal(out=rstd[:], in_=sd[:])
                hn = pC.tile([128, 64], F32, tag="hn")
                nc.vector.tensor_sub(hn[:], h_sb[:], mu[:].to_broadcast([128, 64]))
                nc.vector.tensor_mul(hn[:], hn[:], rstd[:].to_broadcast([128, 64]))
                nc.vector.tensor_mul(hn[:], hn[:], lng_sb[:])
                nc.vector.tensor_add(hn[:], hn[:], lnb_sb[:])
                # tanh-approx gelu composed from primitives (matches jax default)
                u3 = pC.tile([128, 64], F32, tag="u3")
                nc.vector.tensor_mul(u3[:], hn[:], hn[:])
                nc.vector.tensor_mul(u3[:], u3[:], hn[:])
                nc.vector.scalar_tensor_tensor(out=u3[:], in0=u3[:], scalar=0.044715,
                                               in1=hn[:], op0=mybir.AluOpType.mult,
                                               op1=mybir.AluOpType.add)
                th = pC.tile([128, 64], F32, tag="th")
                nc.scalar.activation(out=th[:], in_=u3[:],
                                     func=mybir.ActivationFunctionType.Tanh,
                                     scale=float(np.sqrt(2.0 / np.pi)))
                hg = pC.tile([128, 64], F32, tag="hg")
                nc.vector.tensor_scalar(out=hg[:], in0=hn[:], scalar1=0.5,
                                        scalar2=None, op0=mybir.AluOpType.mult)
                nc.vector.scalar_tensor_tensor(out=hg[:], in0=th[:], scalar=1.0,
                                               in1=hg[:], op0=mybir.AluOpType.add,
                                               op1=mybir.AluOpType.mult)
                # second MLP layer: transpose hg, matmul with Wo2
                nc.tensor.transpose(out=ps_t, in_=hg[:], identity=ident_sb[:])
                hgT = pC.tile([64, 128], F32, tag="hgT")
                nc.vector.tensor_copy(out=hgT[:], in_=ps_t)
                nc.tensor.matmul(out=ps_off, lhsT=hgT[:], rhs=Wo2_sb[:],
                                 start=True, stop=True)
                nc.vector.tensor_add(off_sb[:, k, :].unsqueeze(1),
                                     ps_off.unsqueeze(1),
                                     bo2_sb[:].unsqueeze(1))
                # queries (scaled by 1/sqrt(dh) via host-side W), cast to bf16
                for ch in range(2):
                    nc.tensor.matmul(out=ps_q, lhsT=qfT_sb[:, ch, k * 128:(k + 1) * 128],
                                     rhs=Wqs_sb[:, ch, :], start=(ch == 0), stop=(ch == 1))
                nc.scalar.copy(out=q_sb[:, k, :], in_=ps_q)

        # patch gather source AP: step unit 512 elems, element 1024 elems (2KB)
        G_patches = bass.AP(tensor=Gt, offset=0, ap=[[UNIT, 2 * B_BASE], [1, 2 * UNIT]])
        if debug:
            nc.sync.dma_start(out=dbg["qfT"][:].rearrange("c p n -> p c n"), in_=qfT_sb[:])
            nc.sync.dma_start(out=dbg["off"][:], in_=off_sb[:])
            dq = main.tile([128, NG, C], F32)
            nc.vector.tensor_copy(dq[:], q_sb[:])
            nc.sync.dma_start(out=dbg["q"][:], in_=dq[:])

        # ---- phase D: coords, weights, gather indices ----
        # layouts: [128 r, 64] with free index = g*8 + p
        w4all = main.tile([128, 256], BF16)        # col = (g*8+p)*4 + wi
        Ridx = main.tile([128, 512], I16)          # [(g,p,t)] wrapped idx, 8x replicated
        with tc.tile_pool(name="pD", bufs=1) as pD, \
             tc.tile_pool(name="pD_ps", bufs=2, space="PSUM") as pD_ps:
            x = pD.tile([128, 64], F32)
            y = pD.tile([128, 64], F32)
            offx = bass.AP(tensor=off_sb[:].tensor, offset=off_sb[:].offset,
                           ap=[off_sb[:].ap[0], [16, NG], [2, NP]])
            offy = bass.AP(tensor=off_sb[:].tensor, offset=off_sb[:].offset + 1,
                           ap=[off_sb[:].ap[0], [16, NG], [2, NP]])
            nc.vector.tensor_add(x[:], offx, refx_sb[:].to_broadcast([128, 64]))
            refy_pg = bass.AP(tensor=refy_sb[:].tensor, offset=refy_sb[:].offset,
                              ap=[refy_sb[:].ap[0], [1, NG], [0, NP]])
            nc.vector.tensor_add(y[:], offy, refy_pg)
            if debug:
                dxy = pD.tile([128, 2, 64], F32)
                nc.vector.tensor_copy(dxy[:, 0, :], x[:])
                nc.vector.tensor_copy(dxy[:, 1, :], y[:])
                nc.sync.dma_start(out=dbg["xy"][:], in_=dxy[:])

            def floor_pos(v, dst):
                """dst = floor(v) for any-rounding int casts."""
                vi = pD.tile([128, 64], I32, tag="fc_i")
                nc.vector.tensor_copy(out=vi[:], in_=v[:])
                nc.vector.tensor_copy(out=dst[:], in_=vi[:])
                gt = pD.tile([128, 64], F32, tag="fc_g")
                nc.vector.tensor_tensor(out=gt[:], in0=dst[:], in1=v[:],
                                        op=mybir.AluOpType.is_gt)
                nc.vector.tensor_sub(dst[:], dst[:], gt[:])

            def clip01(v):
                nc.vector.tensor_scalar(out=v[:], in0=v[:], scalar1=0.0, scalar2=1.0,
                                        op0=mybir.AluOpType.max,
                                        op1=mybir.AluOpType.min)

            x0c = pD.tile([128, 64], F32); wx = pD.tile([128, 64], F32)
            y0c = pD.tile([128, 64], F32); wy = pD.tile([128, 64], F32)
            floor_pos(x, x0c)
            nc.vector.tensor_scalar(out=x0c[:], in0=x0c[:], scalar1=0.0, scalar2=float(W - 2),
                                    op0=mybir.AluOpType.max, op1=mybir.AluOpType.min)
            nc.vector.tensor_sub(wx[:], x[:], x0c[:]); clip01(wx)
            floor_pos(y, y0c)
            nc.vector.tensor_scalar(out=y0c[:], in0=y0c[:], scalar1=0.0, scalar2=float(H - 2),
                                    op0=mybir.AluOpType.max, op1=mybir.AluOpType.min)
            nc.vector.tensor_sub(wy[:], y[:], y0c[:]); clip01(wy)
            wx1 = pD.tile([128, 64], F32)
            nc.vector.tensor_scalar(out=wx1[:], in0=wx[:], scalar1=-1.0, scalar2=1.0,
                                    op0=mybir.AluOpType.mult, op1=mybir.AluOpType.add)
            wy1 = pD.tile([128, 64], F32)
            nc.vector.tensor_scalar(out=wy1[:], in0=wy[:], scalar1=-1.0, scalar2=1.0,
                                    op0=mybir.AluOpType.mult, op1=mybir.AluOpType.add)

            def w4_slice(wi):
                # column layout (g, p, wi): col = (g*8 + p)*4 + wi
                a = w4all[:]
                return bass.AP(tensor=a.tensor, offset=a.offset + wi, ap=[a.ap[0], [4, 64]])
            # order [w00, w10, w01, w11] to match patch element slices
            nc.vector.tensor_mul(w4_slice(0), wy1[:], wx1[:])
            nc.vector.tensor_mul(w4_slice(1), wy[:], wx1[:])
            nc.vector.tensor_mul(w4_slice(2), wy1[:], wx[:])
            nc.vector.tensor_mul(w4_slice(3), wy[:], wx[:])
            if debug:
                dw4 = pD.tile([128, 256], F32)
                nc.vector.tensor_copy(dw4[:], w4all[:])
                nc.sync.dma_start(out=dbg["w4"][:], in_=dw4[:])

            # patch idx = par*8192 + ((y0-par)/2)*128 + x0
            yh = pD.tile([128, 64], F32)
            half_ = pD.tile([128, 64], F32)
            nc.vector.tensor_scalar(out=half_[:], in0=y0c[:], scalar1=0.5, scalar2=None,
                                    op0=mybir.AluOpType.mult)
            floor_pos(half_, yh)
            par = pD.tile([128, 64], F32)
            nc.vector.tensor_scalar(out=par[:], in0=yh[:], scalar1=-2.0, scalar2=None,
                                    op0=mybir.AluOpType.mult)
            nc.vector.tensor_add(par[:], par[:], y0c[:])
            idxf = pD.tile([128, 64], F32)
            nc.vector.tensor_scalar(out=idxf[:], in0=par[:], scalar1=float(B_BASE),
                                    scalar2=None, op0=mybir.AluOpType.mult)
            nc.vector.tensor_scalar(out=yh[:], in0=yh[:], scalar1=128.0, scalar2=None,
                                    op0=mybir.AluOpType.mult)
            nc.vector.tensor_add(idxf[:], idxf[:], yh[:])
            nc.vector.tensor_add(idxf[:], idxf[:], x0c[:])
            if debug:
                nc.sync.dma_start(out=dbg["idxf"][:], in_=idxf[:])

            # rearrange idx into wrapped [16, (g,p,t)] layout (8x partition-replicated)
            Rf = pD.tile([128, 512], F32)
            for t in range(8):
                ps_r = pD_ps.tile([128, 64], F32, tag="ps_r")
                nc.tensor.matmul(out=ps_r[:], lhsT=fold8_sb[:, t * 128:(t + 1) * 128],
                                 rhs=idxf[:], start=True, stop=True)
                dst = bass.AP(tensor=Rf[:].tensor, offset=Rf[:].offset + t,
                              ap=[Rf[:].ap[0], [8, 64]])
                nc.vector.tensor_copy(out=dst, in_=ps_r[:])
            nc.vector.tensor_copy(out=Ridx[:], in_=Rf[:])

        # ---- phase E+F (fused, g-major): gather, PE diag-transpose-combine,
        #      K/V matmuls, stacked scores, softmax, attn*V ----
        out_sb = main.tile([128, NG, C], F32)
        with tc.tile_pool(name="pE_raw", bufs=4) as pE_raw, \
             tc.tile_pool(name="pE", bufs=4) as pE, \
             tc.tile_pool(name="pF", bufs=3) as pF, \
             tc.tile_pool(name="pE_ps", bufs=2, space="PSUM") as pE_ps, \
             tc.tile_pool(name="pE_ps_kv", bufs=3, space="PSUM") as pE_ps_kv:
            for g in range(NG):
                patch = pE_raw.tile([128, NP, 1024], BF16, tag="patch")
                # two half-gathers: the first 4 points' combine matmuls can
                # start after half the drain latency
                nc.gpsimd.dma_gather(patch[:, 0:4, :], G_patches,
                                     Ridx[:, g * 64:g * 64 + 32],
                                     NQ // 2, NQ // 2, 1024, elem_step=UNIT)
                nc.gpsimd.dma_gather(patch[:, 4:8, :], G_patches,
                                     Ridx[:, g * 64 + 32:(g + 1) * 64],
                                     NQ // 2, NQ // 2, 1024, elem_step=UNIT)
                kv_g = pF.tile([128, 2, NP, C], BF16, tag="kv_g")
                for pp in range(NP // 2):      # point pairs
                    p0 = 2 * pp
                    # diag[r, m*4+wi] = (r==m) * w4all[r, (g*8+p)*4+wi];
                    # built per point pair, packed wi quad -> DVE 2x mode
                    diag2 = pE.tile([128, 2, 512], BF16, tag="diag2")
                    wsl = bass.AP(tensor=w4all[:].tensor,
                                  offset=w4all[:].offset + (g * 8 + p0) * 4,
                                  ap=[w4all[:].ap[0], [4, 2], [0, 128], [1, 4]])
                    i4b = bass.AP(tensor=ident4_sb[:].tensor,
                                  offset=ident4_sb[:].offset,
                                  ap=[ident4_sb[:].ap[0], [0, 2], [1, 512]])
                    nc.vector.tensor_tensor(out=diag2[:], in0=i4b, in1=wsl,
                                            op=mybir.AluOpType.mult)
                    ps_sT = pE_ps.tile([128, 2, 2, 128], F32, tag="ps_sT")
                    for sp in range(2):
                        for ch in range(2):
                            for wi in range(4):
                                dslice = bass.AP(tensor=diag2[:].tensor,
                                                 offset=diag2[:].offset + sp * 512 + wi,
                                                 ap=[diag2[:].ap[0], [4, 128]])
                                nc.tensor.matmul(
                                    out=ps_sT[:, sp, ch, :],
                                    lhsT=patch[:, p0 + sp, wi * 256 + ch * 128: wi * 256 + ch * 128 + 128],
                                    rhs=dslice,
                                    start=(wi == 0), stop=(wi == 3),
                                )
                    sT = pE.tile([128, 2, 2, 128], BF16, tag="sT")
                    nc.scalar.copy(out=sT[:], in_=ps_sT[:])
                    ps_kv = pE_ps_kv.tile([128, 2, 512], F32, tag="ps_kv")
                    for sp in range(2):
                        for ch in range(2):
                            nc.tensor.matmul(out=ps_kv[:, sp, :], lhsT=sT[:, sp, ch, :],
                                             rhs=WkvT_sb[:, ch, :],
                                             start=(ch == 0), stop=(ch == 1))
                    # paired copy: [kv(2), pt(2), C] <- [pt(2), kv(2)*C]
                    kv_dst = bass.AP(tensor=kv_g[:].tensor,
                                     offset=kv_g[:].offset + p0 * C,
                                     ap=[kv_g[:].ap[0], [NP * C, 2], [C, 2], [1, C]])
                    kv_src = bass.AP(tensor=ps_kv[:].tensor,
                                     offset=ps_kv[:].offset,
                                     ap=[ps_kv[:].ap[0], [C, 2], [2 * C, 2], [1, C]])
                    nc.scalar.copy(out=kv_dst, in_=kv_src)
                # stacked scores: qk over all points at once
                qk = pF.tile([128, NP, C], BF16, tag="qk")
                q_b = bass.AP(tensor=q_sb[:].tensor,
                              offset=q_sb[:].offset + g * C,
                              ap=[q_sb[:].ap[0], [0, NP], [1, C]])
                nc.vector.tensor_mul(qk[:], kv_g[:, 0, :, :], q_b)
                qk2 = pF.tile([128, NP, NHEAD, 16], BF16, tag="qk2")
                qkv_ = qk[:].rearrange("r p (h s d) -> r (p h s) d", h=NHEAD, s=2)
                with nc.allow_low_precision(reason="bf16 partial sums of 16 products"):
                    nc.vector.tensor_tensor(
                        out=qk2[:],
                        in0=bass.AP(tensor=qk[:].tensor, offset=qk[:].offset,
                                    ap=[qk[:].ap[0], [DH, NP * NHEAD], [1, 16]]),
                        in1=bass.AP(tensor=qk[:].tensor, offset=qk[:].offset + 16,
                                    ap=[qk[:].ap[0], [DH, NP * NHEAD], [1, 16]]),
                        op=mybir.AluOpType.add)
                scores_g = pF.tile([128, NP, NHEAD], BF16, tag="scores_g")
                with nc.allow_low_precision(reason="bf16 scores: f32 accum, one rounding"):
                    nc.vector.tensor_reduce(
                        out=scores_g[:],
                        in_=qk2[:].rearrange("r p h d -> r (p h) d"),
                        axis=mybir.AxisListType.X, op=mybir.AluOpType.add)
                # softmax over p
                mx = pF.tile([128, NHEAD], BF16, tag="mx")
                s_hp = bass.AP(tensor=scores_g[:].tensor, offset=scores_g[:].offset,
                               ap=[scores_g[:].ap[0], [1, NHEAD], [NHEAD, NP]])
                nc.vector.tensor_reduce(out=mx[:], in_=s_hp,
                                        axis=mybir.AxisListType.X,
                                        op=mybir.AluOpType.max)
                e = pF.tile([128, NP, NHEAD], F32, tag="e")
                mxb = bass.AP(tensor=mx[:].tensor, offset=mx[:].offset,
                              ap=[mx[:].ap[0], [0, NP], [1, NHEAD]])
                nc.vector.tensor_sub(e[:], scores_g[:], mxb)
                nc.scalar.activation(out=e[:], in_=e[:],
                                     func=mybir.ActivationFunctionType.Exp)
                s1 = pF.tile([128, NHEAD], F32, tag="s1")
                e_hp = bass.AP(tensor=e[:].tensor, offset=e[:].offset,
                               ap=[e[:].ap[0], [1, NHEAD], [NHEAD, NP]])
                nc.vector.tensor_reduce(out=s1[:], in_=e_hp,
                                        axis=mybir.AxisListType.X,
                                        op=mybir.AluOpType.add)
                rs = pF.tile([128, NHEAD], F32, tag="rs")
                nc.vector.reciprocal(out=rs[:], in_=s1[:])
                attn = pF.tile([128, NP, NHEAD], BF16, tag="attn")
                rsb = bass.AP(tensor=rs[:].tensor, offset=rs[:].offset,
                              ap=[rs[:].ap[0], [0, NP], [1, NHEAD]])
                nc.vector.tensor_mul(attn[:], e[:], rsb)
                av = pF.tile([128, NP, C], BF16, tag="av")
                attn_b = bass.AP(tensor=attn[:].tensor, offset=attn[:].offset,
                                 ap=[attn[:].ap[0], [NHEAD, NP], [1, NHEAD], [0, DH]])
                nc.vector.tensor_tensor(out=av[:], in0=attn_b, in1=kv_g[:, 1, :, :],
                                        op=mybir.AluOpType.mult)
                # tree-sum over the 8 points (contiguous bf16 adds)
                t4 = pF.tile([128, 4, C], BF16, tag="t4")
                nc.vector.tensor_add(t4[:], av[:, 0:4, :], av[:, 4:8, :])
                t2 = pF.tile([128, 2, C], BF16, tag="t2")
                nc.vector.tensor_add(t2[:], t4[:, 0:2, :], t4[:, 2:4, :])
                nc.vector.tensor_add(out_sb[:, g, :].unsqueeze(1), t2[:, 0:1, :], t2[:, 1:2, :])
                nc.sync.dma_start(
                    out=out.rearrange("(gg r) c -> r gg c", gg=NG)[:, g, :].unsqueeze(1),
                    in_=out_sb[:, g, :].unsqueeze(1),
                )
        if debug:
            pass

    return nc


_CACHE = {}


def _get_nc(debug=False):
    key = ("nc", debug)
    if key not in _CACHE:
        nc = build_nc(debug)
        nc.compile()
        _CACHE[key] = nc
    return _CACHE[key]


def make_in_maps(feature_map, W_q, W_k, W_v, W_o1, b_o1, ln_g, ln_b, W_o2, b_o2):
    B = feature_map.shape[0]
    consts = _host_constants()
    shared = dict(
        Wqs=np.ascontiguousarray(W_q.T) / np.float32(np.sqrt(DH)),
        WkvT=np.ascontiguousarray(np.concatenate([W_k.T, W_v.T], axis=1)).astype(_BF),
        Wo1=np.ascontiguousarray(W_o1),
        Wo2s=np.ascontiguousarray(W_o2) * np.float32(4.0),
        bo1b=np.tile(b_o1[None, :], (128, 1)).astype(np.float32),
        bo2b=np.tile(b_o2[None, :] * np.float32(4.0), (128, 1)).astype(np.float32),
        lngb=np.tile(ln_g[None, :], (128, 1)).astype(np.float32),
        lnbb=np.tile(ln_b[None, :], (128, 1)).astype(np.float32),
        ident=consts["ident"], ident4=consts["ident4"],
        fold8=consts["fold8"], SXw=consts["SXw"],
        refx=consts["refx"], refy=consts["refy"],
    )
    in_maps = []
    for b in range(B):
        m = dict(shared)
        m["fm"] = np.ascontiguousarray(feature_map[b].reshape(C, HW))
        in_maps.append(m)
    return in_maps


def kernel(**inputs):
    from concourse.bass_utils import run_bass_kernel_spmd
    nc = _get_nc()
    in_maps = make_in_maps(**inputs)
    B = len(in_maps)
    res = run_bass_kernel_spmd(nc, in_maps, list(range(B)))
    out = np.stack([res.results[b]["out"] for b in range(B)], axis=0)
    return out.astype(np.float32)


# revision 14
# speedup vs baseline: 1.3166x; 1.3166x over previous
"""Deformable attention module on Trainium2 (Bass/Tile), 8-core data-parallel.

v2 strategy (per core = one batch):
  1. Phase A (8 chunks of 16 y-rows): load fm [C=256, HW] f32 -> SBUF;
     PE-transpose 128x128 blocks, cast bf16; write TWO y-banded copies of
     the transposed map to DRAM (A: bands (2b,2b+1), B: (2b+1,2b+2)) so a
     2x2 bilinear patch is ONE contiguous 2KB gather element.  Fused into
     the same loop: q_featT = bilinear sample at the fixed ref grid,
     computed as PE selection-matmuls (fmT_chunk^T @ SXw) accumulating the
     two y-rows of each ref row in PSUM -> no separate q-feat gather, no
     DRAM round trip, available the moment the last chunk is transposed.
  2. Offset MLP batched over all 8 query blocks (PE matmuls, manual
     layernorm, composed tanh-gelu) + queries -> overlaps phase A's banded
     write drain.
  3. Coordinates -> robust floor -> clip -> bilinear weights + banded patch
     indices (int16); identity-slice PE matmuls wrap indices into the
     dma_gather layout.
  4. Per query-block g: one 2-half dma_gather of 1024 2KB patches (GPSIMD
     runs only descriptor generation, so gathers pipeline across blocks).
     Per point: DVE builds the bilinear diagonal, PE diag-transpose-combine
     -> sT [C, 128] bf16, K/V matmuls -> interleaved K/V planes.  Scores:
     stacked q*k multiply + two-stage segmented reduce.  Softmax, attn*V on
     DVE + bf16 tree-sum -> out.
"""

import sys

for _p in ("/opt/trn_rl_repo", "/root/.axon_site/_ro/trn_rl_repo"):
    if _p not in sys.path:
        sys.path.append(_p)

import numpy as np
import ml_dtypes

import concourse.bass as bass
import concourse.bacc as bacc
import concourse.tile as tile
from concourse import mybir

F32 = mybir.dt.float32
BF16 = mybir.dt.bfloat16
I16 = mybir.dt.int16
I32 = mybir.dt.int32

C = 256
H = W = 128
HW = H * W
NQ = 1024          # (H//4) * (W//4)
NHEAD = 8
DH = 32
NG = 8             # query blocks of 128
NP = 8             # sampling points per query (= NHEAD)

# banded gather source: "unit" = 512 bf16 = 2 image rows; element = 2 units
UNIT = 512                 # bf16 elems per step unit
B_BASE = 8192              # B copy starts at unit 8192
G_UNITS = 16448            # 16384 + pad

_BF = ml_dtypes.bfloat16


def _ref_grids():
    """Per-ref-point pixel coords / floor / weights, matching reference.py fp32 math."""
    c = np.linspace(-1.0, 1.0, 32).astype(np.float32)
    pix = ((c + 1.0) * 0.5 * (W - 1)).astype(np.float32)   # [32]
    p0 = np.clip(np.floor(pix), 0.0, W - 2).astype(np.float32)
    wf = np.clip(pix - p0, 0.0, 1.0).astype(np.float32)
    return pix, p0, wf


def _host_constants():
    pix, p0, wf = _ref_grids()

    # selection matmul rhs: SXw[x, (gy_local, row), gx] built per 16-row
    # chunk; all chunks share the same (x0, wx) column structure and the
    # same (gy_local -> wy) pattern EXCEPT wy varies per gy, so store the
    # full [128, 32*2, 32] table indexed by (gy, row).
    x0 = p0.astype(np.int32)
    wx = wf
    y0 = p0.astype(np.int32)          # same grid in y
    wy = wf
    SXw = np.zeros((128, 64, 32), np.float32)
    for gy in range(32):
        for r in range(2):
            wyfac = (1.0 - wy[gy]) if r == 0 else wy[gy]
            for gx in range(32):
                SXw[x0[gx], gy * 2 + r, gx] += wyfac * (1.0 - wx[gx])
                SXw[x0[gx] + 1, gy * 2 + r, gx] += wyfac * wx[gx]
    SXw = SXw.astype(_BF)

    ident = np.eye(128, dtype=np.float32)
    # interleaved identity: ident4[r, m*4+wi] = (r == m), so the diag build's
    # weight operand has a packed (stride-1) last dim -> DVE 2x mode
    ident4 = np.repeat(np.eye(128, dtype=np.float32), 4, axis=1).astype(_BF)  # [128,512]
    # fold8[r, t, m] = 1 iff r == t*16 + (m % 16): the identity-slice matmul
    # then emits idx row block t into all 8 replicated 16-partition groups.
    fold8 = np.zeros((128, 8, 128), np.float32)
    for t in range(8):
        for m in range(128):
            fold8[t * 16 + (m % 16), t, m] = 1.0
    fold8 = fold8.reshape(128, 1024)

    # ref pixel coords in the [128 r, 8 g] layout (n = g*128 + r)
    refx = pix[np.arange(128) % 32].astype(np.float32)[:, None]          # [128,1]
    g_idx, r_idx = np.meshgrid(np.arange(NG), np.arange(128), indexing="xy")
    refy = pix[(g_idx * 4 + r_idx // 32)].astype(np.float32)             # [128,8]
    return dict(
        ident=ident, ident4=ident4, fold8=fold8, SXw=SXw,
        refx=refx, refy=refy,
    )


def build_nc(debug: bool = False):
    nc = bacc.Bacc()

    fm = nc.declare_dram_parameter("fm", [C, HW], F32, isOutput=False)
    Wqs = nc.declare_dram_parameter("Wqs", [C, C], F32, isOutput=False)
    WkvT = nc.declare_dram_parameter("WkvT", [C, 2 * C], BF16, isOutput=False)
    Wo1 = nc.declare_dram_parameter("Wo1", [C, 64], F32, isOutput=False)
    Wo2s = nc.declare_dram_parameter("Wo2s", [64, 16], F32, isOutput=False)
    bo1b = nc.declare_dram_parameter("bo1b", [128, 64], F32, isOutput=False)
    bo2b = nc.declare_dram_parameter("bo2b", [128, 16], F32, isOutput=False)
    lngb = nc.declare_dram_parameter("lngb", [128, 64], F32, isOutput=False)
    lnbb = nc.declare_dram_parameter("lnbb", [128, 64], F32, isOutput=False)
    identP = nc.declare_dram_parameter("ident", [128, 128], F32, isOutput=False)
    ident4P = nc.declare_dram_parameter("ident4", [128, 512], BF16, isOutput=False)
    SXwP = nc.declare_dram_parameter("SXw", [128, 64, 32], BF16, isOutput=False)
    refxP = nc.declare_dram_parameter("refx", [128, 1], F32, isOutput=False)
    refyP = nc.declare_dram_parameter("refy", [128, NG], F32, isOutput=False)
    fold8P = nc.declare_dram_parameter("fold8", [128, 1024], F32, isOutput=False)

    out = nc.declare_dram_parameter("out", [NQ, C], F32, isOutput=True)
    dbg = {}
    if debug:
        dbg["qfT"] = nc.declare_dram_parameter("d_qfT", [2, 128, NQ], F32, isOutput=True)
        dbg["off"] = nc.declare_dram_parameter("d_off", [128, NG, 16], F32, isOutput=True)
        dbg["xy"] = nc.declare_dram_parameter("d_xy", [128, 2, 64], F32, isOutput=True)
        dbg["w4"] = nc.declare_dram_parameter("d_w4", [128, 256], F32, isOutput=True)
        dbg["idxf"] = nc.declare_dram_parameter("d_idxf", [128, 64], F32, isOutput=True)
        dbg["q"] = nc.declare_dram_parameter("d_q", [128, NG, C], F32, isOutput=True)

    with tile.TileContext(nc) as tc, tc.tile_pool(name="main", bufs=1) as main, \
         tc.tile_pool(name="consts", bufs=1) as consts, \
         tc.tile_pool(name="dram", bufs=1, space="DRAM") as dram:

        # ---- constants to SBUF (scalar queue; sync reserved for fm loads) ----
        ident_sb = consts.tile([128, 128], F32)
        nc.scalar.dma_start(out=ident_sb[:], in_=identP[:])
        SXw_sb = consts.tile([128, 64, 32], BF16)
        nc.scalar.dma_start(out=SXw_sb[:], in_=SXwP[:])
        Wo1_sb = consts.tile([128, 2, 64], F32)
        nc.scalar.dma_start(out=Wo1_sb[:], in_=Wo1.rearrange("(ch k) d -> k ch d", ch=2))
        Wqs_sb = consts.tile([128, 2, C], F32)
        nc.scalar.dma_start(out=Wqs_sb[:], in_=Wqs.rearrange("(ch k) d -> k ch d", ch=2))
        bo1_sb = consts.tile([128, 64], F32)
        nc.scalar.dma_start(out=bo1_sb[:], in_=bo1b[:])
        bo2_sb = consts.tile([128, 16], F32)
        nc.scalar.dma_start(out=bo2_sb[:], in_=bo2b[:])
        lng_sb = consts.tile([128, 64], F32)
        nc.scalar.dma_start(out=lng_sb[:], in_=lngb[:])
        lnb_sb = consts.tile([128, 64], F32)
        nc.scalar.dma_start(out=lnb_sb[:], in_=lnbb[:])
        Wo2_sb = consts.tile([64, 16], F32)
        nc.scalar.dma_start(out=Wo2_sb[:], in_=Wo2s[:])
        ident4_sb = consts.tile([128, 512], BF16)
        nc.scalar.dma_start(out=ident4_sb[:], in_=ident4P[:])
        fold8_sb = consts.tile([128, 1024], F32)
        nc.scalar.dma_start(out=fold8_sb[:], in_=fold8P[:])
        refx_sb = consts.tile([128, 1], F32)
        nc.scalar.dma_start(out=refx_sb[:], in_=refxP[:])
        refy_sb = consts.tile([128, NG], F32)
        nc.scalar.dma_start(out=refy_sb[:], in_=refyP[:])
        WkvT_sb = consts.tile([128, 2, 2 * C], BF16)
        nc.scalar.dma_start(out=WkvT_sb[:], in_=WkvT.rearrange("(ch k) d -> k ch d", ch=2))
        eps_sb = consts.tile([128, 1], F32)
        nc.vector.memset(eps_sb[:], 1e-5)
        # preload activation tables (Sqrt/Tanh/Exp) off the critical path
        tdummy = consts.tile([128, 1], F32)
        nc.vector.memset(tdummy[:], 1.0)
        for fn in (mybir.ActivationFunctionType.Sqrt,
                   mybir.ActivationFunctionType.Tanh,
                   mybir.ActivationFunctionType.Exp):
            nc.scalar.activation(out=tdummy[:], in_=tdummy[:], func=fn)

        # ---- phase A: fm -> banded transposed copies A/B in DRAM,
        #      fused q_featT selection matmuls ----
        G = dram.tile([G_UNITS, UNIT], BF16)
        Gt = G[:].tensor

        def g_ap(unit_off, elem_off, dims):
            return bass.AP(tensor=Gt, offset=unit_off * UNIT + elem_off, ap=list(dims))

        qfT_sb = main.tile([128, 2, NQ], F32)      # [c_lo/c_hi part, ch, n]
        q_sb = main.tile([128, NG, C], BF16)
        off_sb = main.tile([128, NG, 16], F32)
        _, p0_grid, _ = _ref_grids()
        with tc.tile_pool(name="pA_fm", bufs=2) as pA_fm, \
             tc.tile_pool(name="pA_ps", bufs=3, space="PSUM") as pA_ps, \
             tc.tile_pool(name="pA_psq", bufs=2, space="PSUM") as pA_psq, \
             tc.tile_pool(name="pC_ps", bufs=2, space="PSUM") as pC_ps, \
             tc.tile_pool(name="pC", bufs=2) as pC, \
             tc.tile_pool(name="pA_out", bufs=3) as pA_out:
            # zero-fill regions the banded writes never touch (B band 63
            # upper halves + tail pad) so the gather source is fully defined
            zpad = pA_out.tile([128, UNIT], BF16, tag="zpad")
            nc.vector.memset(zpad[:], 0.0)
            nc.gpsimd.dma_start(out=G[:][B_BASE + 8064:B_BASE + 8192, C:2 * C],
                                in_=zpad[:, 0:C])
            nc.gpsimd.dma_start(out=G[:][2 * B_BASE:2 * B_BASE + 64, :],
                                in_=zpad[0:64, :])
            def emit_chunk(k):
                fm_sb = pA_fm.tile([128, 2, HW // 8], F32, tag="fm_sb")
                hw0 = k * (HW // 8)
                nc.sync.dma_start(out=fm_sb[:, 0, :], in_=fm[0:128, hw0:hw0 + HW // 8])
                nc.sync.dma_start(out=fm_sb[:, 1, :], in_=fm[128:256, hw0:hw0 + HW // 8])
                fmT_sb = pA_out.tile([128, 16, C], BF16, tag="fmT_sb")
                for jj in range(16):       # y-row within this chunk
                    if jj % 2 == 0:
                        ps = pA_ps.tile([128, 512], F32, tag="psA")
                    for ch in range(2):
                        nc.tensor.transpose(
                            out=ps[:, (jj % 2) * 256 + ch * 128:(jj % 2) * 256 + ch * 128 + 128],
                            in_=fm_sb[:, ch, jj * 128:(jj + 1) * 128],
                            identity=ident_sb[:],
                        )
                    if jj % 2 == 1:
                        if (jj // 2) % 2 == 0:
                            nc.vector.tensor_copy(out=fmT_sb[:, jj - 1:jj + 1, :], in_=ps[:])
                        else:
                            nc.scalar.copy(out=fmT_sb[:, jj - 1:jj + 1, :], in_=ps[:])
                # banded writes for this chunk (scalar + gpsimd queues; sync
                # stays pure-load so chunk k+1 loads are never stuck behind
                # a write waiting on chunk k's transposes)
                band0 = 8 * k                  # first A band of this chunk
                dstA = g_ap(band0 * 128, 0,
                            [[UNIT, 128], [128 * UNIT, 8], [1, 2 * C]])
                srcA = bass.AP(tensor=fmT_sb[:].tensor,
                               offset=fmT_sb[:].offset,
                               ap=[fmT_sb[:].ap[0], [2 * C, 8], [1, 2 * C]])
                nc.scalar.dma_start(out=dstA, in_=srcA)
                dstB = g_ap(B_BASE + band0 * 128, 0,
                            [[UNIT, 128], [128 * UNIT, 7], [1, 2 * C]])
                srcB = bass.AP(tensor=fmT_sb[:].tensor,
                               offset=fmT_sb[:].offset + C,
                               ap=[fmT_sb[:].ap[0], [2 * C, 7], [1, 2 * C]])
                nc.gpsimd.dma_start(out=dstB, in_=srcB)
                dstBt = g_ap(B_BASE + (band0 + 7) * 128, 0,
                             [[UNIT, 128], [1, C]])
                srcBt = bass.AP(tensor=fmT_sb[:].tensor,
                                offset=fmT_sb[:].offset + 15 * C,
                                ap=[fmT_sb[:].ap[0], [1, C]])
                nc.gpsimd.dma_start(out=dstBt, in_=srcBt)
                if band0 > 0:
                    dstBh = g_ap(B_BASE + (band0 - 1) * 128, C,
                                 [[UNIT, 128], [1, C]])
                    srcBh = bass.AP(tensor=fmT_sb[:].tensor,
                                    offset=fmT_sb[:].offset,
                                    ap=[fmT_sb[:].ap[0], [1, C]])
                    nc.gpsimd.dma_start(out=dstBh, in_=srcBh)
                # q_featT selection matmuls: chunk k holds ref rows
                # gy = 4k..4k+3 with y0(gy), y0(gy)+1 inside the chunk.
                psq = pA_psq.tile([128, 2, 128], F32, tag="psq")
                for gl in range(4):
                    gy = 4 * k + gl
                    y0 = int(p0_grid[gy])
                    j0 = y0 - 16 * k
                    for ch in range(2):
                        for r in range(2):
                            nc.tensor.matmul(
                                out=psq[:, ch, gl * 32:(gl + 1) * 32],
                                lhsT=fmT_sb[:, j0 + r, ch * 128:(ch + 1) * 128],
                                rhs=SXw_sb[:, gy * 2 + r, :],
                                start=(r == 0), stop=(r == 1),
                            )
                if k % 2 == 0:
                    nc.vector.tensor_copy(out=qfT_sb[:, :, k * 128:(k + 1) * 128],
                                          in_=psq[:])
                else:
                    nc.scalar.copy(out=qfT_sb[:, :, k * 128:(k + 1) * 128],
                                   in_=psq[:])

            def emit_cblock(k):
                # ---- offset MLP + queries for block k (fused into phase A,
                # issued one chunk late so its vector/scalar queue work never
                # stalls the next chunk's PSUM->SBUF copies) ----
                # one PSUM bank carved into regions: [0:64) ps_h, [64:192)
                # ps_t (64 partitions), [192:208) ps_off, [208:464) ps_q
                pCall = pC_ps.tile([128, 512], F32, tag="ps_c")
                ps_h = pCall[:, 0:64]
                ps_t = pCall[0:64, 64:192]
                ps_off = pCall[:, 192:208]
                ps_q = pCall[:, 208:464]
                for ch in range(2):
                    nc.tensor.matmul(out=ps_h,
                                     lhsT=qfT_sb[:, ch, k * 128:(k + 1) * 128],
                                     rhs=Wo1_sb[:, ch, :],
                                     start=(ch == 0), stop=(ch == 1))
                h_sb = pC.tile([128, 64], F32, tag="h_sb")
                nc.vector.tensor_add(h_sb[:], ps_h, bo1_sb[:])
                # manual layernorm over the last (64) axis
                sq = pC.tile([128, 64], F32, tag="sq")
                nc.vector.tensor_mul(sq[:], h_sb[:], h_sb[:])
                mu = pC.tile([128, 1], F32, tag="mu")
                nc.vector.tensor_reduce(out=mu[:], in_=h_sb[:],
                                        axis=mybir.AxisListType.X, op=mybir.AluOpType.add)
                ex2 = pC.tile([128, 1], F32, tag="ex2")
                nc.vector.tensor_reduce(out=ex2[:], in_=sq[:],
                                        axis=mybir.AxisListType.X, op=mybir.AluOpType.add)
                nc.vector.tensor_scalar(out=mu[:], in0=mu[:], scalar1=1.0 / 64, scalar2=None,
                                        op0=mybir.AluOpType.mult)
                nc.vector.tensor_scalar(out=ex2[:], in0=ex2[:], scalar1=1.0 / 64, scalar2=None,
                                        op0=mybir.AluOpType.mult)
                mu2 = pC.tile([128, 1], F32, tag="mu2")
                nc.vector.tensor_mul(mu2[:], mu[:], mu[:])
                var = pC.tile([128, 1], F32, tag="var")
                nc.vector.tensor_sub(var[:], ex2[:], mu2[:])
                sd = pC.tile([128, 1], F32, tag="sd")
                nc.scalar.activation(out=sd[:], in_=var[:],
                                     func=mybir.ActivationFunctionType.Sqrt,
                                     bias=eps_sb[:])
                rstd = pC.tile([128, 1], F32, tag="rstd")
                nc.vector.reciprocal(out=rstd[:], in_=sd[:])
                hn = pC.tile([128, 64], F32, tag="hn")
                nc.vector.tensor_sub(hn[:], h_sb[:], mu[:].to_broadcast([128, 64]))
                nc.vector.tensor_mul(hn[:], hn[:], rstd[:].to_broadcast([128, 64]))
                nc.vector.tensor_mul(hn[:], hn[:], lng_sb[:])
                nc.vector.tensor_add(hn[:], hn[:], lnb_sb[:])
                # tanh-approx gelu composed from primitives (matches jax default)
                u3 = pC.tile([128, 64], F32, tag="u3")
                nc.vector.tensor_mul(u3[:], hn[:], hn[:])
                nc.vector.tensor_mul(u3[:], u3[:], hn[:])
                nc.vector.scalar_tensor_tensor(out=u3[:], in0=u3[:], scalar=0.044715,
                                               in1=hn[:], op0=mybir.AluOpType.mult,
                                               op1=mybir.AluOpType.add)
                th = pC.tile([128, 64], F32, tag="th")
                nc.scalar.activation(out=th[:], in_=u3[:],
                                     func=mybir.ActivationFunctionType.Tanh,
                                     scale=float(np.sqrt(2.0 / np.pi)))
                hg = pC.tile([128, 64], F32, tag="hg")
                nc.vector.tensor_scalar(out=hg[:], in0=hn[:], scalar1=0.5,
                                        scalar2=None, op0=mybir.AluOpType.mult)
                nc.vector.scalar_tensor_tensor(out=hg[:], in0=th[:], scalar=1.0,
                                               in1=hg[:], op0=mybir.AluOpType.add,
                                               op1=mybir.AluOpType.mult)
                # second MLP layer: transpose hg, matmul with Wo2
                nc.tensor.transpose(out=ps_t, in_=hg[:], identity=ident_sb[:])
                hgT = pC.tile([64, 128], F32, tag="hgT")
                nc.vector.tensor_copy(out=hgT[:], in_=ps_t)
                nc.tensor.matmul(out=ps_off, lhsT=hgT[:], rhs=Wo2_sb[:],
                                 start=True, stop=True)
                nc.vector.tensor_add(off_sb[:, k, :].unsqueeze(1),
                                     ps_off.unsqueeze(1),
                                     bo2_sb[:].unsqueeze(1))
                # queries (scaled by 1/sqrt(dh) via host-side W), cast to bf16
                for ch in range(2):
                    nc.tensor.matmul(out=ps_q, lhsT=qfT_sb[:, ch, k * 128:(k + 1) * 128],
                                     rhs=Wqs_sb[:, ch, :], start=(ch == 0), stop=(ch == 1))
                nc.scalar.copy(out=q_sb[:, k, :], in_=ps_q)

            for k in range(8):
                emit_chunk(k)
                if k >= 1:
                    emit_cblock(k - 1)
            emit_cblock(7)

        # patch gather source AP: step unit 512 elems, element 1024 elems (2KB)
        G_patches = bass.AP(tensor=Gt, offset=0, ap=[[UNIT, 2 * B_BASE], [1, 2 * UNIT]])
        if debug:
            nc.sync.dma_start(out=dbg["qfT"][:].rearrange("c p n -> p c n"), in_=qfT_sb[:])
            nc.sync.dma_start(out=dbg["off"][:], in_=off_sb[:])
            dq = main.tile([128, NG, C], F32)
            nc.vector.tensor_copy(dq[:], q_sb[:])
            nc.sync.dma_start(out=dbg["q"][:], in_=dq[:])

        # ---- phase D: coords, weights, gather indices ----
        # layouts: [128 r, 64] with free index = g*8 + p
        w4all = main.tile([128, 256], BF16)        # col = (g*8+p)*4 + wi
        Ridx = main.tile([128, 512], I16)          # [(g,p,t)] wrapped idx, 8x replicated
        with tc.tile_pool(name="pD", bufs=1) as pD, \
             tc.tile_pool(name="pD_ps", bufs=2, space="PSUM") as pD_ps:
            x = pD.tile([128, 64], F32)
            y = pD.tile([128, 64], F32)
            offx = bass.AP(tensor=off_sb[:].tensor, offset=off_sb[:].offset,
                           ap=[off_sb[:].ap[0], [16, NG], [2, NP]])
            offy = bass.AP(tensor=off_sb[:].tensor, offset=off_sb[:].offset + 1,
                           ap=[off_sb[:].ap[0], [16, NG], [2, NP]])
            nc.vector.tensor_add(x[:], offx, refx_sb[:].to_broadcast([128, 64]))
            refy_pg = bass.AP(tensor=refy_sb[:].tensor, offset=refy_sb[:].offset,
                              ap=[refy_sb[:].ap[0], [1, NG], [0, NP]])
            nc.vector.tensor_add(y[:], offy, refy_pg)
            if debug:
                dxy = pD.tile([128, 2, 64], F32)
                nc.vector.tensor_copy(dxy[:, 0, :], x[:])
                nc.vector.tensor_copy(dxy[:, 1, :], y[:])
                nc.sync.dma_start(out=dbg["xy"][:], in_=dxy[:])

            def floor_pos(v, dst):
                """dst = floor(v) for any-rounding int casts."""
                vi = pD.tile([128, 64], I32, tag="fc_i")
                nc.vector.tensor_copy(out=vi[:], in_=v[:])
                nc.vector.tensor_copy(out=dst[:], in_=vi[:])
                gt = pD.tile([128, 64], F32, tag="fc_g")
                nc.vector.tensor_tensor(out=gt[:], in0=dst[:], in1=v[:],
                                        op=mybir.AluOpType.is_gt)
                nc.vector.tensor_sub(dst[:], dst[:], gt[:])

            def clip01(v):
                nc.vector.tensor_scalar(out=v[:], in0=v[:], scalar1=0.0, scalar2=1.0,
                                        op0=mybir.AluOpType.max,
                                        op1=mybir.AluOpType.min)

            x0c = pD.tile([128, 64], F32); wx = pD.tile([128, 64], F32)
            y0c = pD.tile([128, 64], F32); wy = pD.tile([128, 64], F32)
            floor_pos(x, x0c)
            nc.vector.tensor_scalar(out=x0c[:], in0=x0c[:], scalar1=0.0, scalar2=float(W - 2),
                                    op0=mybir.AluOpType.max, op1=mybir.AluOpType.min)
            nc.vector.tensor_sub(wx[:], x[:], x0c[:]); clip01(wx)
            floor_pos(y, y0c)
            nc.vector.tensor_scalar(out=y0c[:], in0=y0c[:], scalar1=0.0, scalar2=float(H - 2),
                                    op0=mybir.AluOpType.max, op1=mybir.AluOpType.min)
            nc.vector.tensor_sub(wy[:], y[:], y0c[:]); clip01(wy)
            wx1 = pD.tile([128, 64], F32)
            nc.vector.tensor_scalar(out=wx1[:], in0=wx[:], scalar1=-1.0, scalar2=1.0,
                                    op0=mybir.AluOpType.mult, op1=mybir.AluOpType.add)
            wy1 = pD.tile([128, 64], F32)
            nc.vector.tensor_scalar(out=wy1[:], in0=wy[:], scalar1=-1.0, scalar2=1.0,
                                    op0=mybir.AluOpType.mult, op1=mybir.AluOpType.add)

            def w4_slice(wi):
                # column layout (g, p, wi): col = (g*8 + p)*4 + wi
                a = w4all[:]
                return bass.AP(tensor=a.tensor, offset=a.offset + wi, ap=[a.ap[0], [4, 64]])
            # order [w00, w10, w01, w11] to match patch element slices
            nc.vector.tensor_mul(w4_slice(0), wy1[:], wx1[:])
            nc.vector.tensor_mul(w4_slice(1), wy[:], wx1[:])
            nc.vector.tensor_mul(w4_slice(2), wy1[:], wx[:])
            nc.vector.tensor_mul(w4_slice(3), wy[:], wx[:])
            if debug:
                dw4 = pD.tile([128, 256], F32)
                nc.vector.tensor_copy(dw4[:], w4all[:])
                nc.sync.dma_start(out=dbg["w4"][:], in_=dw4[:])

            # patch idx = par*8192 + ((y0-par)/2)*128 + x0
            yh = pD.tile([128, 64], F32)
            half_ = pD.tile([128, 64], F32)
            nc.vector.tensor_scalar(out=half_[:], in0=y0c[:], scalar1=0.5, scalar2=None,
                                    op0=mybir.AluOpType.mult)
            floor_pos(half_, yh)
            par = pD.tile([128, 64], F32)
            nc.vector.tensor_scalar(out=par[:], in0=yh[:], scalar1=-2.0, scalar2=None,
                                    op0=mybir.AluOpType.mult)
            nc.vector.tensor_add(par[:], par[:], y0c[:])
            idxf = pD.tile([128, 64], F32)
            nc.vector.tensor_scalar(out=idxf[:], in0=par[:], scalar1=float(B_BASE),
                                    scalar2=None, op0=mybir.AluOpType.mult)
            nc.vector.tensor_scalar(out=yh[:], in0=yh[:], scalar1=128.0, scalar2=None,
                                    op0=mybir.AluOpType.mult)
            nc.vector.tensor_add(idxf[:], idxf[:], yh[:])
            nc.vector.tensor_add(idxf[:], idxf[:], x0c[:])
            if debug:
                nc.sync.dma_start(out=dbg["idxf"][:], in_=idxf[:])

            # rearrange idx into wrapped [16, (g,p,t)] layout (8x partition-replicated)
            Rf = pD.tile([128, 512], F32)
            for t in range(8):
                ps_r = pD_ps.tile([128, 64], F32, tag="ps_r")
                nc.tensor.matmul(out=ps_r[:], lhsT=fold8_sb[:, t * 128:(t + 1) * 128],
                                 rhs=idxf[:], start=True, stop=True)
                dst = bass.AP(tensor=Rf[:].tensor, offset=Rf[:].offset + t,
                              ap=[Rf[:].ap[0], [8, 64]])
                nc.vector.tensor_copy(out=dst, in_=ps_r[:])
            nc.vector.tensor_copy(out=Ridx[:], in_=Rf[:])

        # ---- phase E+F (fused, g-major): gather, PE diag-transpose-combine,
        #      K/V matmuls, stacked scores, softmax, attn*V ----
        out_sb = main.tile([128, NG, C], F32)
        with tc.tile_pool(name="pE_raw", bufs=4) as pE_raw, \
             tc.tile_pool(name="pE", bufs=4) as pE, \
             tc.tile_pool(name="pF", bufs=3) as pF, \
             tc.tile_pool(name="pE_ps", bufs=2, space="PSUM") as pE_ps, \
             tc.tile_pool(name="pE_ps_kv", bufs=3, space="PSUM") as pE_ps_kv:
            for g in range(NG):
                patch = pE_raw.tile([128, NP, 1024], BF16, tag="patch")
                # two half-gathers: the first 4 points' combine matmuls can
                # start after half the drain latency
                nc.gpsimd.dma_gather(patch[:, 0:4, :], G_patches,
                                     Ridx[:, g * 64:g * 64 + 32],
                                     NQ // 2, NQ // 2, 1024, elem_step=UNIT)
                nc.gpsimd.dma_gather(patch[:, 4:8, :], G_patches,
                                     Ridx[:, g * 64 + 32:(g + 1) * 64],
                                     NQ // 2, NQ // 2, 1024, elem_step=UNIT)
                kv_g = pF.tile([128, 2, NP, C], BF16, tag="kv_g")
                for pp in range(NP // 2):      # point pairs
                    p0 = 2 * pp
                    # diag[r, m*4+wi] = (r==m) * w4all[r, (g*8+p)*4+wi];
                    # built per point pair, packed wi quad -> DVE 2x mode
                    diag2 = pE.tile([128, 2, 512], BF16, tag="diag2")
                    wsl = bass.AP(tensor=w4all[:].tensor,
                                  offset=w4all[:].offset + (g * 8 + p0) * 4,
                                  ap=[w4all[:].ap[0], [4, 2], [0, 128], [1, 4]])
                    i4b = bass.AP(tensor=ident4_sb[:].tensor,
                                  offset=ident4_sb[:].offset,
                                  ap=[ident4_sb[:].ap[0], [0, 2], [1, 512]])
                    nc.vector.tensor_tensor(out=diag2[:], in0=i4b, in1=wsl,
                                            op=mybir.AluOpType.mult)
                    ps_sT = pE_ps.tile([128, 2, 2, 128], F32, tag="ps_sT")
                    for sp in range(2):
                        for ch in range(2):
                            for wi in range(4):
                                dslice = bass.AP(tensor=diag2[:].tensor,
                                                 offset=diag2[:].offset + sp * 512 + wi,
                                                 ap=[diag2[:].ap[0], [4, 128]])
                                nc.tensor.matmul(
                                    out=ps_sT[:, sp, ch, :],
                                    lhsT=patch[:, p0 + sp, wi * 256 + ch * 128: wi * 256 + ch * 128 + 128],
                                    rhs=dslice,
                                    start=(wi == 0), stop=(wi == 3),
                                )
                    sT = pE.tile([128, 2, 2, 128], BF16, tag="sT")
                    nc.scalar.copy(out=sT[:], in_=ps_sT[:])
                    ps_kv = pE_ps_kv.tile([128, 2, 512], F32, tag="ps_kv")
                    for sp in range(2):
                        for ch in range(2):
                            nc.tensor.matmul(out=ps_kv[:, sp, :], lhsT=sT[:, sp, ch, :],
                                             rhs=WkvT_sb[:, ch, :],
                                             start=(ch == 0), stop=(ch == 1))
                    # paired copy: [kv(2), pt(2), C] <- [pt(2), kv(2)*C]
                    kv_dst = bass.AP(tensor=kv_g[:].tensor,
                                     offset=kv_g[:].offset + p0 * C,
                                     ap=[kv_g[:].ap[0], [NP * C, 2], [C, 2], [1, C]])
                    kv_src = bass.AP(tensor=ps_kv[:].tensor,
                                     offset=ps_kv[:].offset,
                                     ap=[ps_kv[:].ap[0], [C, 2], [2 * C, 2], [1, C]])
                    nc.scalar.copy(out=kv_dst, in_=kv_src)
                # stacked scores: qk over all points at once
                qk = pF.tile([128, NP, C], BF16, tag="qk")
                q_b = bass.AP(tensor=q_sb[:].tensor,
                              offset=q_sb[:].offset + g * C,
                              ap=[q_sb[:].ap[0], [0, NP], [1, C]])
                nc.vector.tensor_mul(qk[:], kv_g[:, 0, :, :], q_b)
                qk2 = pF.tile([128, NP, NHEAD, 16], BF16, tag="qk2")
                qkv_ = qk[:].rearrange("r p (h s d) -> r (p h s) d", h=NHEAD, s=2)
                with nc.allow_low_precision(reason="bf16 partial sums of 16 products"):
                    nc.vector.tensor_tensor(
                        out=qk2[:],
                        in0=bass.AP(tensor=qk[:].tensor, offset=qk[:].offset,
                                    ap=[qk[:].ap[0], [DH, NP * NHEAD], [1, 16]]),
                        in1=bass.AP(tensor=qk[:].tensor, offset=qk[:].offset + 16,
                                    ap=[qk[:].ap[0], [DH, NP * NHEAD], [1, 16]]),
                        op=mybir.AluOpType.add)
                scores_g = pF.tile([128, NP, NHEAD], BF16, tag="scores_g")
                with nc.allow_low_precision(reason="bf16 scores: f32 accum, one rounding"):
                    nc.vector.tensor_reduce(
                        out=scores_g[:],
                        in_=qk2[:].rearrange("r p h d -> r (p h) d"),
                        axis=mybir.AxisListType.X, op=mybir.AluOpType.add)
                # softmax over p
                mx = pF.tile([128, NHEAD], BF16, tag="mx")
                s_hp = bass.AP(tensor=scores_g[:].tensor, offset=scores_g[:].offset,
                               ap=[scores_g[:].ap[0], [1, NHEAD], [NHEAD, NP]])
                nc.vector.tensor_reduce(out=mx[:], in_=s_hp,
                                        axis=mybir.AxisListType.X,
                                        op=mybir.AluOpType.max)
                e = pF.tile([128, NP, NHEAD], F32, tag="e")
                mxb = bass.AP(tensor=mx[:].tensor, offset=mx[:].offset,
                              ap=[mx[:].ap[0], [0, NP], [1, NHEAD]])
                nc.vector.tensor_sub(e[:], scores_g[:], mxb)
                nc.scalar.activation(out=e[:], in_=e[:],
                                     func=mybir.ActivationFunctionType.Exp)
                s1 = pF.tile([128, NHEAD], F32, tag="s1")
                e_hp = bass.AP(tensor=e[:].tensor, offset=e[:].offset,
                               ap=[e[:].ap[0], [1, NHEAD], [NHEAD, NP]])
                nc.vector.tensor_reduce(out=s1[:], in_=e_hp,
                                        axis=mybir.AxisListType.X,
                                        op=mybir.AluOpType.add)
                rs = pF.tile([128, NHEAD], F32, tag="rs")
                nc.vector.reciprocal(out=rs[:], in_=s1[:])
                attn = pF.tile([128, NP, NHEAD], BF16, tag="attn")
                rsb = bass.AP(tensor=rs[:].tensor, offset=rs[:].offset,
                              ap=[rs[:].ap[0], [0, NP], [1, NHEAD]])
                nc.vector.tensor_mul(attn[:], e[:], rsb)
                av = pF.tile([128, NP, C], BF16, tag="av")
                attn_b = bass.AP(tensor=attn[:].tensor, offset=attn[:].offset,
                                 ap=[attn[:].ap[0], [NHEAD, NP], [1, NHEAD], [0, DH]])
                nc.vector.tensor_tensor(out=av[:], in0=attn_b, in1=kv_g[:, 1, :, :],
                                        op=mybir.AluOpType.mult)
                # tree-sum over the 8 points (contiguous bf16 adds)
                t4 = pF.tile([128, 4, C], BF16, tag="t4")
                nc.vector.tensor_add(t4[:], av[:, 0:4, :], av[:, 4:8, :])
                t2 = pF.tile([128, 2, C], BF16, tag="t2")
                nc.vector.tensor_add(t2[:], t4[:, 0:2, :], t4[:, 2:4, :])
                nc.vector.tensor_add(out_sb[:, g, :].unsqueeze(1), t2[:, 0:1, :], t2[:, 1:2, :])
                nc.sync.dma_start(
                    out=out.rearrange("(gg r) c -> r gg c", gg=NG)[:, g, :].unsqueeze(1),
                    in_=out_sb[:, g, :].unsqueeze(1),
                )
        if debug:
            pass

    return nc


_CACHE = {}


def _get_nc(debug=False):
    key = ("nc", debug)
    if key not in _CACHE:
        nc = build_nc(debug)
        nc.compile()
        _CACHE[key] = nc
    return _CACHE[key]


def make_in_maps(feature_map, W_q, W_k, W_v, W_o1, b_o1, ln_g, ln_b, W_o2, b_o2):
    B = feature_map.shape[0]
    consts = _host_constants()
    shared = dict(
        Wqs=np.ascontiguousarray(W_q.T) / np.float32(np.sqrt(DH)),
        WkvT=np.ascontiguousarray(np.concatenate([W_k.T, W_v.T], axis=1)).astype(_BF),
        Wo1=np.ascontiguousarray(W_o1),
        Wo2s=np.ascontiguousarray(W_o2) * np.float32(4.0),
        bo1b=np.tile(b_o1[None, :], (128, 1)).astype(np.float32),
        bo2b=np.tile(b_o2[None, :] * np.float32(4.0), (128, 1)).astype(np.float32),
        lngb=np.tile(ln_g[None, :], (128, 1)).astype(np.float32),
        lnbb=np.tile(ln_b[None, :], (128, 1)).astype(np.float32),
        ident=consts["ident"], ident4=consts["ident4"],
        fold8=consts["fold8"], SXw=consts["SXw"],
        refx=consts["refx"], refy=consts["refy"],
    )
    in_maps = []
    for b in range(B):
        m = dict(shared)
        m["fm"] = np.ascontiguousarray(feature_map[b].reshape(C, HW))
        in_maps.append(m)
    return in_maps


def kernel(**inputs):
    from concourse.bass_utils import run_bass_kernel_spmd
    nc = _get_nc()
    in_maps = make_in_maps(**inputs)
    B = len(in_maps)
    res = run_bass_kernel_spmd(nc, in_maps, list(range(B)))
    out = np.stack([res.results[b]["out"] for b in range(B)], axis=0)
    return out.astype(np.float32)
